# revision 6
# baseline (speedup 1.0000x reference)
"""Trainium2 Bass kernel for nn_CustomLayerMKM: y = x @ kron(W2, W1).T + bias.

x: (8, 8192, 1024) fp32, W1/W2: (32, 32), bias: (1024,).
Data-parallel over the 65536 tokens across 8 NeuronCores; weights replicated.

On-device algorithm (default, _build_bd): per 128-token chunk, reshape each
token to X (32x32) and compute Y = W2 @ X @ W1.T using block-diagonal
128x128 matmuls (lhsT = kron(I4, W.T), float32r at 1 cyc/row) between PE
transpose stages that move data token-major <-> feature-major. A dense-K
fallback (_build) materializes kron(W2, W1) on host.
"""

import functools
import numpy as np

B, S, IN, OUT = 8, 8192, 1024, 1024
N_CORES = 8
TOKENS = B * S
TOK_PER_CORE = TOKENS // N_CORES  # 8192
SUP = 512  # tokens per superblock


@functools.lru_cache(maxsize=4)
def _build(n_tok=TOK_PER_CORE, use_f32r=True, reps=1):
    import concourse.bass as bass  # noqa: F401
    import concourse.tile as tile
    from concourse import bacc, mybir
    from concourse.masks import make_identity
    from contextlib import ExitStack

    f32 = mybir.dt.float32
    mmdt = mybir.dt.float32r if use_f32r else f32

    assert n_tok % SUP == 0
    nc = bacc.Bacc("TRN2", target_bir_lowering=False, debug=False,
                   num_devices=N_CORES)
    x = nc.dram_tensor("x", [n_tok, IN], f32, kind="ExternalInput").ap()
    # kt[p, (kb*8+m)*128 + i] = K.T[kb*128+p, m*128+i]  (host-prepared)
    kt = nc.dram_tensor("kt", [128, 8192], mmdt, kind="ExternalInput").ap()
    bb = nc.dram_tensor("bias_bcast", [128, OUT], f32, kind="ExternalInput").ap()
    y = nc.dram_tensor("y", [n_tok, OUT], f32, kind="ExternalOutput").ap()

    with tile.TileContext(nc) as tc, ExitStack() as ctx:
        const = ctx.enter_context(tc.tile_pool(name="const", bufs=1))
        xpool = ctx.enter_context(tc.tile_pool(name="xin", bufs=2))
        xtpool = ctx.enter_context(tc.tile_pool(name="xt", bufs=2))
        ypool = ctx.enter_context(tc.tile_pool(name="ysb", bufs=2))
        ytpool = ctx.enter_context(tc.tile_pool(name="ytok", bufs=2))
        ps_in = ctx.enter_context(tc.tile_pool(name="ps_in", bufs=2, space="PSUM"))
        ps_mm = ctx.enter_context(tc.tile_pool(name="ps_mm", bufs=2, space="PSUM"))
        ps_out = ctx.enter_context(tc.tile_pool(name="ps_out", bufs=2, space="PSUM"))

        ident = const.tile([128, 128], f32)
        make_identity(nc, ident[:])
        ktile = const.tile([128, 8192], mmdt)
        nc.sync.dma_start(ktile[:], kt[:, :])
        btile = const.tile([128, OUT], f32)
        nc.sync.dma_start(btile[:], bb[:, :])

        def body():
            for sb in range(n_tok // SUP):
                one_superblock(sb)

        def one_superblock(sb):
            r0 = sb * SUP
            # ---- load 512 tokens: SBUF [p=tok%128, free=(a, f)] ----
            xin = xpool.tile([128, 4 * IN], f32)
            nc.sync.dma_start(
                xin[:].rearrange("p (a f) -> p a f", a=4),
                x[r0:r0 + SUP, :].rearrange("(a p) f -> p a f", p=128))
            # ---- T-in: feature-major XT [p = f%128, free=(kb, a, tq)] ----
            xt_sb = xtpool.tile([128, 4096], mmdt)
            for fb in range(8):
                pin = ps_in.tile([128, 512], f32)
                for a in range(4):
                    nc.tensor.transpose(
                        pin[:, a * 128:(a + 1) * 128],
                        xin[:, a * IN + fb * 128: a * IN + (fb + 1) * 128],
                        ident[:])
                nc.scalar.copy(xt_sb[:, fb * 512:(fb + 1) * 512], pin[:])
            # ---- dense matmul: y_sb [p=i%128, free=(m, a, tq)] ----
            y_sb = ypool.tile([128, 4096], f32)
            for m in range(8):
                pm = ps_mm.tile([128, 512], f32)
                for kb in range(8):
                    nc.tensor.matmul(
                        pm[:],
                        ktile[:, (kb * 8 + m) * 128:(kb * 8 + m + 1) * 128],
                        xt_sb[:, kb * 512:(kb + 1) * 512],
                        start=(kb == 0), stop=(kb == 7))
                nc.scalar.copy(y_sb[:, m * 512:(m + 1) * 512], pm[:])
            # ---- T-out + bias: ytok [p=tok%128, free=(a, i)] ----
            yt = ytpool.tile([128, 4 * OUT], f32)
            for a in range(4):
                pot = ps_out.tile([128, 1024], f32)
                for m in range(8):
                    nc.tensor.transpose(
                        pot[:, m * 128:(m + 1) * 128],
                        y_sb[:, m * 512 + a * 128: m * 512 + (a + 1) * 128],
                        ident[:])
                nc.vector.tensor_add(
                    yt[:, a * OUT:(a + 1) * OUT], pot[:], btile[:])
            nc.sync.dma_start(
                y[r0:r0 + SUP, :].rearrange("(a p) f -> p a f", p=128),
                yt[:].rearrange("p (a f) -> p a f", a=4))

        if reps == 1:
            body()
        else:
            with tc.For_i(0, reps, 1):
                body()

    nc.compile()
    return nc


@functools.lru_cache(maxsize=6)
def _build_bd(n_tok=TOK_PER_CORE, mode="f32r", reps=1):
    """Block-diagonal factored kernel: MM stages are full 128x128 matmuls with
    lhsT = kron(I4, W.T), processing 4 consecutive j2 (resp. i1) per call.
    Unlike tile_position col-tiling this is f32r-eligible (1 cyc/row at N>=256).

    mode: "f32r" (x/z rounded to f32r at the two MM inputs, rest fp32),
          "f32" (exact), "bf16" (everything bf16 on chip).
    """
    import concourse.bass as bass  # noqa: F401
    import concourse.tile as tile
    from concourse import bacc, mybir
    from concourse.masks import make_identity
    from contextlib import ExitStack

    f32 = mybir.dt.float32
    mmdt = {"f32": f32, "f32r": mybir.dt.float32r,
            "bf16": mybir.dt.bfloat16}[mode]
    flowdt = mybir.dt.bfloat16 if mode == "bf16" else f32

    assert n_tok % SUP == 0
    nc = bacc.Bacc("TRN2", target_bir_lowering=False, debug=False,
                   num_devices=N_CORES)
    x = nc.dram_tensor("x", [n_tok, IN], f32, kind="ExternalInput").ap()
    w1bd = nc.dram_tensor("w1bd", [128, 128], mmdt, kind="ExternalInput").ap()
    w2bd = nc.dram_tensor("w2bd", [128, 128], mmdt, kind="ExternalInput").ap()
    bb = nc.dram_tensor("bias_bcast", [128, OUT], f32, kind="ExternalInput").ap()
    y = nc.dram_tensor("y", [n_tok, OUT], f32, kind="ExternalOutput").ap()

    with tile.TileContext(nc) as tc, ExitStack() as ctx:
        const = ctx.enter_context(tc.tile_pool(name="const", bufs=1))
        xpool = ctx.enter_context(tc.tile_pool(name="xin", bufs=2))
        xtpool = ctx.enter_context(tc.tile_pool(name="xt", bufs=2))
        ztokp = ctx.enter_context(tc.tile_pool(name="ztok", bufs=2))
        ztsbp = ctx.enter_context(tc.tile_pool(name="ztsb", bufs=1))
        ytokp = ctx.enter_context(tc.tile_pool(name="ytok", bufs=2))
        ps_tA = ctx.enter_context(tc.tile_pool(name="ps_tA", bufs=3, space="PSUM"))
        ps_tB = ctx.enter_context(tc.tile_pool(name="ps_tB", bufs=5, space="PSUM"))

        ident = const.tile([128, 128], f32)
        make_identity(nc, ident[:])
        identf = const.tile([128, 128], flowdt)
        make_identity(nc, identf[:])
        w1tt = const.tile([128, 128], mmdt)
        nc.sync.dma_start(w1tt[:], w1bd[:, :])
        w2tt = const.tile([128, 128], mmdt)
        nc.sync.dma_start(w2tt[:], w2bd[:, :])
        btile = const.tile([128, OUT], f32)
        nc.sync.dma_start(btile[:], bb[:, :])

        def one_superblock(sb):
            r0 = sb * SUP
            xin = xpool.tile([128, 4 * IN], f32)
            nc.sync.dma_start(
                xin[:].rearrange("p (a f) -> p a f", a=4),
                x[r0:r0 + SUP, :].rearrange("(a p) f -> p a f", p=128))
            # ---- T-in: XT [p=(b,j1), free=(g, a, tq)], dtype mmdt ----
            xt_sb = xtpool.tile([128, 4096], mmdt)
            for g in range(8):
                pin = ps_tA.tile([128, 512], f32, name="pin", tag="tA")
                for a in range(4):
                    nc.tensor.transpose(
                        pin[:, a * 128:(a + 1) * 128],
                        xin[:, a * IN + g * 128: a * IN + (g + 1) * 128],
                        ident[:])
                nc.scalar.copy(xt_sb[:, g * 512:(g + 1) * 512], pin[:])

            yt = ytokp.tile([128, 4 * OUT], f32)
            # ---- fused MM1+T-mid: one matmul per (g,k): lhsT = XT-slice
            # (stationary), rhs = w1bd -> out = Z.T block [t, (b,i1)];
            # zf = i1*32 + j2, j2 = 4*g+b = 16*p0+4*gg+b ----
            zt_k = [ztokp.tile([128, 1024], flowdt, name=f"ztk{k}",
                               tag=f"ztok{k}")
                    for k in range(4)]
            for p0 in range(2):
                for k in range(4):
                    tm = ps_tB.tile([128, 512], f32, name="tm", tag="tB")
                    for gg in range(4):
                        g = 4 * p0 + gg
                        nc.tensor.matmul(
                            tm[:, gg * 128:(gg + 1) * 128],
                            xt_sb[:, g * 512 + k * 128:
                                  g * 512 + k * 128 + 128],
                            w1tt[:],
                            start=True, stop=True)
                    dest = zt_k[k][:].rearrange(
                        "p (i1 po gg b) -> p po gg b i1",
                        i1=32, po=2, gg=4, b=4)[:, p0:p0 + 1]
                    src = tm[:].rearrange(
                        "p (u gg b i1) -> p u gg b i1", u=1, gg=4, b=4, i1=32)
                    nc.vector.tensor_copy(dest, src)
            # ---- T-in2: ZT [p=(d,j2), (h, k, t)], dtype mmdt ----
            zt_sb = ztsbp.tile([128, 4096], mmdt)
            for k in range(4):
                for hp in range(2):
                    ti2 = ps_tA.tile([128, 512], flowdt, name="ti2", tag="tA")
                    for hh in range(4):
                        h = 4 * hp + hh
                        nc.tensor.transpose(
                            ti2[:, hh * 128:(hh + 1) * 128],
                            zt_k[k][:, h * 128:(h + 1) * 128],
                            identf[:])
                    dest = zt_sb[:].rearrange(
                        "p (h k t) -> p h k t", h=8, k=4, t=128
                    )[:, 4 * hp:4 * hp + 4, k:k + 1]
                    src = ti2[:].rearrange(
                        "p (h u t) -> p h u t", h=4, u=1, t=128)
                    nc.scalar.copy(dest, src)
            # ---- fused MM2+T-out: lhsT = ZT-slice, rhs = w2bd ->
            # out = Y.T block [t, (d,i2)]; yf = i2*32+16*q0+4*hh+d ----
            for q0 in range(2):
                for k in range(4):
                    to = ps_tB.tile([128, 512], f32, name="to", tag="tB")
                    for hh in range(4):
                        h = 4 * q0 + hh
                        nc.tensor.matmul(
                            to[:, hh * 128:(hh + 1) * 128],
                            zt_sb[:, h * 512 + k * 128:
                                  h * 512 + k * 128 + 128],
                            w2tt[:],
                            start=True, stop=True)
                    dest = yt[:, k * OUT:(k + 1) * OUT].rearrange(
                        "p (i2 q hh d) -> p q hh d i2",
                        i2=32, q=2, hh=4, d=4)[:, q0:q0 + 1]
                    bsrc = btile[:].rearrange(
                        "p (i2 q hh d) -> p q hh d i2",
                        i2=32, q=2, hh=4, d=4)[:, q0:q0 + 1]
                    src = to[:].rearrange(
                        "p (u hh d i2) -> p u hh d i2",
                        u=1, hh=4, d=4, i2=32)
                    nc.vector.tensor_add(dest, src, bsrc)
            nc.sync.dma_start(
                y[r0:r0 + SUP, :].rearrange("(a p) f -> p a f", p=128),
                yt[:].rearrange("p (a f) -> p a f", a=4))

        def body():
            for sb in range(n_tok // SUP):
                one_superblock(sb)

        if reps == 1:
            body()
        else:
            with tc.For_i(0, reps, 1):
                body()

    nc.compile()
    return nc


SUPV2 = 1024  # tokens per superblock in v2


@functools.lru_cache(maxsize=6)
def _build_v2(n_tok=TOK_PER_CORE, reps=1):
    """v2: host-packed bf16 layout, transpose-free 2-matmul pipeline.

    Host packs x so each 128x128 SBUF chunk is [p=(tl,j2), f=(g,j1)],
    token t = sb*1024 + k*16 + g*4 + tl, feature f = j2*32 + j1.
    MM1: lhsT = x chunk (stationary), rhs = kron(I4, W2.T) (moving)
         -> out[p=(g,j1), f=(tl,i2)] = V = W2 @ X_t  (already "transposed")
    MM2: lhsT = kron(I4, W1.T) (stationary), rhs = V (bf16)
         -> out[p=(g,i1), f=(tl,i2)] = W2 @ X_t @ W1.T
    Bias is added on host after unpack.
    """
    import concourse.bass as bass  # noqa: F401
    import concourse.tile as tile
    from concourse import bacc, mybir
    from contextlib import ExitStack

    f32 = mybir.dt.float32
    bf16 = mybir.dt.bfloat16

    assert n_tok % SUPV2 == 0
    n_sb = n_tok // SUPV2
    FD = SUPV2 * 8  # free-dim bf16 elems per superblock row block (8192)
    nc = bacc.Bacc("TRN2", target_bir_lowering=False, debug=False,
                   num_devices=N_CORES)
    x = nc.dram_tensor("x", [n_sb * 128, FD], bf16, kind="ExternalInput").ap()
    w2m = nc.dram_tensor("w2m", [128, 128], bf16, kind="ExternalInput").ap()
    w1s = nc.dram_tensor("w1s", [128, 128], bf16, kind="ExternalInput").ap()
    y = nc.dram_tensor("y", [n_sb * 128, FD], bf16, kind="ExternalOutput").ap()

    with tile.TileContext(nc) as tc, ExitStack() as ctx:
        const = ctx.enter_context(tc.tile_pool(name="const", bufs=1))
        xpool = ctx.enter_context(tc.tile_pool(name="xin", bufs=3))
        vpool = ctx.enter_context(tc.tile_pool(name="vmid", bufs=4))
        ypool = ctx.enter_context(tc.tile_pool(name="ysb", bufs=2))
        ps1 = ctx.enter_context(tc.tile_pool(name="ps1", bufs=3, space="PSUM"))
        ps2 = ctx.enter_context(tc.tile_pool(name="ps2", bufs=3, space="PSUM"))

        w2t = const.tile([128, 128], bf16)
        nc.sync.dma_start(w2t[:], w2m[:, :])
        w1t = const.tile([128, 128], bf16)
        nc.sync.dma_start(w1t[:], w1s[:, :])

        NQ = SUPV2 // 64  # 16 quads (of 64 tokens) per superblock
        LAG = 2  # quads between MM1 emission and MM2 emission (hides copy1)

        def body():
            xins = {}
            ysbs = {}
            pending = []  # (sb, q, vsb) awaiting MM2

            def drain_one():
                psb, pq, pv = pending.pop(0)
                po = ps2.tile([128, 512], f32, name="po", tag="ps2")
                nc.tensor.matmul(po[:], w1t[:], pv[:], start=True, stop=True)
                if psb not in ysbs:
                    ysbs[psb] = ypool.tile([128, FD], bf16, name="ysb", tag="ysb")
                nc.vector.tensor_copy(
                    ysbs[psb][:, pq * 512:(pq + 1) * 512], po[:])
                if pq == NQ - 1:
                    nc.sync.dma_start(
                        y[psb * 128:(psb + 1) * 128, :], ysbs.pop(psb)[:])

            def load(sb):
                if sb < n_sb and sb not in xins:
                    xins[sb] = xpool.tile([128, FD], bf16, name="xin", tag="xin")
                    nc.sync.dma_start(
                        xins[sb][:], x[sb * 128:(sb + 1) * 128, :])

            load(0)
            load(1)
            for qi in range(n_sb * NQ):
                sb, q = divmod(qi, NQ)
                if q == 0:
                    load(sb + 2)
                xin = xins[sb]
                pm = ps1.tile([128, 512], f32, name="pm", tag="ps1")
                for kk in range(4):
                    k = 4 * q + kk
                    nc.tensor.matmul(
                        pm[:, kk * 128:(kk + 1) * 128],
                        xin[:, k * 128:(k + 1) * 128],
                        w2t[:],
                        start=True, stop=True)
                vsb = vpool.tile([128, 512], bf16, name="vsb", tag="vmid")
                nc.scalar.copy(vsb[:], pm[:])
                pending.append((sb, q, vsb))
                if q == NQ - 1:
                    xins.pop(sb)
                if len(pending) > LAG:
                    drain_one()
            while pending:
                drain_one()

        if reps == 1:
            body()
        else:
            with tc.For_i(0, reps, 1):
                body()

    nc.compile()
    return nc


def _pack_x_v2(xf, n_tok):
    """xf: (N_CORES*n_tok, 1024) fp32 -> list of per-core [n_sb*128, 8192]
    bf16 arrays with row = sb*128 + tl*32 + j2, col = k*128 + g*32 + j1."""
    import ml_dtypes
    n_sb = n_tok // SUPV2
    xb = xf.astype(ml_dtypes.bfloat16)
    x8 = xb.reshape(N_CORES, n_sb, SUPV2 // 16, 4, 4, 32, 32)
    # [c, s, k, g, tl, j2, j1] -> [c, s, tl, j2, k, g, j1]
    xp = x8.transpose(0, 1, 4, 5, 2, 3, 6)
    return np.ascontiguousarray(xp).reshape(N_CORES, n_sb * 128, SUPV2 * 8)


def _unpack_y_v2(y_cores, bias, n_tok):
    """y_cores: list of [n_sb*128, 8192] bf16, row = sb*128 + g*32 + i1,
    col = q*512 + kk*128 + tl*32 + i2; token = sb*1024 + (q*4+kk)*16 + g*4+tl,
    feature = i2*32 + i1. Returns (N_CORES*n_tok, 1024) fp32 with bias."""
    n_sb = n_tok // SUPV2
    ya = np.stack(y_cores, axis=0).reshape(
        N_CORES, n_sb, 4, 32, SUPV2 // 64, 4, 4, 32)
    # [c, s, g, i1, q, kk, tl, i2] -> [c, s, q, kk, g, tl, i2, i1]
    yt = ya.transpose(0, 1, 4, 5, 2, 6, 7, 3)
    out = np.ascontiguousarray(yt).reshape(N_CORES * n_tok, OUT)
    return out.astype(np.float32) + np.asarray(bias, dtype=np.float32)


def _prep_weights_v2(weight_1, weight_2):
    import ml_dtypes
    w1 = np.asarray(weight_1, dtype=np.float32)
    w2 = np.asarray(weight_2, dtype=np.float32)
    eye4 = np.eye(4, dtype=np.float32)
    w2m = np.ascontiguousarray(np.kron(eye4, w2.T).astype(ml_dtypes.bfloat16))
    w1s = np.ascontiguousarray(np.kron(eye4, w1.T).astype(ml_dtypes.bfloat16))
    return w2m, w1s


def _prep_weights_bd(weight_1, weight_2, bias, mode):
    import ml_dtypes
    w1 = np.asarray(weight_1, dtype=np.float32)
    w2 = np.asarray(weight_2, dtype=np.float32)
    b = np.asarray(bias, dtype=np.float32)
    wdt = ml_dtypes.bfloat16 if mode == "bf16" else np.float32
    eye4 = np.eye(4, dtype=np.float32)
    w1bd = np.ascontiguousarray(np.kron(eye4, w1.T).astype(wdt))
    w2bd = np.ascontiguousarray(np.kron(eye4, w2.T).astype(wdt))
    bias_bcast = np.ascontiguousarray(np.broadcast_to(b, (128, OUT)))
    return w1bd, w2bd, bias_bcast


def _prep_weights(weight_1, weight_2, bias):
    w1 = np.asarray(weight_1, dtype=np.float32)
    w2 = np.asarray(weight_2, dtype=np.float32)
    b = np.asarray(bias, dtype=np.float32)
    K = np.kron(w2, w1)  # (OUT, IN)
    KT = np.ascontiguousarray(K.T)  # (IN, OUT); lhsT[f, i] = K[i, f]
    # kt_host[p, (kb*8+m)*128+i] = KT[kb*128+p, m*128+i]
    kt_host = np.ascontiguousarray(
        KT.reshape(8, 128, 8, 128).transpose(1, 0, 2, 3).reshape(128, 8 * 1024))
    bias_bcast = np.ascontiguousarray(np.broadcast_to(b, (128, OUT)))
    return kt_host, bias_bcast


LAST_RESULTS = None


def kernel(x, weight_1, weight_2, bias, _n_tok=TOK_PER_CORE, _mode="v2",
           _reps=1, _trace=False):
    """_mode: "v2" (host-packed bf16, transpose-free), "bd_f32r" | "bd_f32" |
    "bd_bf16" (block-diag factored) or "dense" / "dense_f32r" (dense-K)."""
    global LAST_RESULTS
    from concourse import bass_utils

    xf = np.ascontiguousarray(np.asarray(x, dtype=np.float32).reshape(-1, IN))
    assert xf.shape[0] == _n_tok * N_CORES, (xf.shape, _n_tok)

    if _mode == "v2":
        w2m, w1s = _prep_weights_v2(weight_1, weight_2)
        x_dev = _pack_x_v2(xf, _n_tok)
        nc = _build_v2(_n_tok, _reps)
        in_maps = [{"x": x_dev[i], "w2m": w2m, "w1s": w1s}
                   for i in range(N_CORES)]
        res = bass_utils.run_bass_kernel_spmd(
            nc, in_maps, core_ids=list(range(N_CORES)), trace=_trace)
        LAST_RESULTS = res
        out = _unpack_y_v2([res.results[i]["y"] for i in range(N_CORES)],
                           bias, _n_tok)
        if _n_tok == TOK_PER_CORE:
            out = out.reshape(B, S, OUT)
        return out

    if _mode in ("dense", "dense_f32r"):
        kt_host, bias_bcast = _prep_weights(weight_1, weight_2, bias)
        nc = _build(_n_tok, _mode == "dense_f32r", _reps)
        wmap = {"kt": kt_host, "bias_bcast": bias_bcast}
    else:
        assert _mode.startswith("bd_"), _mode
        base = _mode[3:]
        w1bd, w2bd, bias_bcast = _prep_weights_bd(
            weight_1, weight_2, bias, base)
        nc = _build_bd(_n_tok, base, _reps)
        wmap = {"w1bd": w1bd, "w2bd": w2bd, "bias_bcast": bias_bcast}

    in_maps = [
        {"x": np.ascontiguousarray(xf[i * _n_tok:(i + 1) * _n_tok]), **wmap}
        for i in range(N_CORES)
    ]
    res = bass_utils.run_bass_kernel_spmd(
        nc, in_maps, core_ids=list(range(N_CORES)), trace=_trace)
    LAST_RESULTS = res
    out = np.concatenate([res.results[i]["y"] for i in range(N_CORES)], axis=0)
    if _n_tok == TOK_PER_CORE:
        out = out.reshape(B, S, OUT)
    return out

